# revision 24
# baseline (speedup 1.0000x reference)
"""Trainium2 Bass kernel for nn_AttentionOperation_32521492365427.

kernel(**inputs) -> np.ndarray, full shapes:
  query/key/value: [8, 8, 64, 1024] f32; gamma_sim/beta_sim: [8];
  gamma_val/beta_val: [512]; output: [8, 512, 1024] f32.

Sharded by HEAD across the 8 NeuronCores (one head per core): both
BatchNorms then have core-local statistics, so there are no collectives.

Per-core math:
 - softmax is shift-invariant => the sim-BN reduces to one per-head scale
   s = gamma_sim / sqrt(var(logits) + EPS); beta/mean drop out (the
   mean^2 term of the variance is O(1e-8) of E[l^2] and is dropped).
 - sumsq(logits_b) = <Gq_b, Gk_b> over 64x64 Gram matrices; the grams of
   batch pairs (2p, 2p+1) are block-diagonal-packed into [128,128]
   matmuls (half the instructions of per-batch grams).
 - PV stationary is [128_m, 64 v | 1 ones]: PV row 64 is the softmax
   denominator; normalize = DVE copy+reciprocal of the den row, gpsimd
   partition-broadcast, DVE multiply straight out of PSUM.
 - val-BN affine + exact (erf) gelu fuse into one ACT pass per batch
   pair; s is broadcast to 128 partitions via a tiny ones-matmul.

Scheduling:
 - PE stream: warmup ramp -> pair-grams -> QK prefill -> bridge matmuls
   over the s-chain latency -> software-pipelined [QK(s+2), PV(s)]
   slots. The tensor engine clock (HAM) needs ~3us of continuous
   activity for 2.4 GHz; idle gaps halve it to 1.2 GHz.
 - logits/pt live as [128,512] half-tiles (lg pool bufs=4 = 4 PSUM
   banks, pv double-buffered = 4 banks): the exp->QK write-after-read
   chain is half-granular so it never serializes the slot pipeline.
 - exp: ACT table exp for chunks {0,1,3,5,7}, DVE Schraudolph bf16-bits
   exp for {2,4,6} (logits reach +-43 sigma, so bf16 range is required).
 - per-batch drains (copy/recip/bcast/mul/bn_stats) pipeline inside the
   next batch's slots; odd batches drain to a staging tile that an idle
   DMA engine lifts to partitions 64-127, so gelu + output stores run
   128 partitions wide (full SDMA fanout) per batch pair. The last
   batch drains in quarter-granular pipelined steps.
 - gelu ACT table preloads via a dummy op gated on the BN aggregate;
   output DMAs alternate between the SP and ACT HWDGE queues.
"""

import os
import sys

sys.path.insert(0, "/opt/trn_rl_repo")

from contextlib import ExitStack

import numpy as np

import concourse.bacc as bacc
import concourse.bass as bass  # noqa: F401
import concourse.tile as tile
from concourse import mybir

F32 = mybir.dt.float32
F16 = mybir.dt.float16
BF16 = mybir.dt.bfloat16
I32 = mybir.dt.int32
I16 = mybir.dt.int16
AF = mybir.ActivationFunctionType
OP = mybir.AluOpType

EPS = 1e-3
NB = 8
D = 64
C = 64
L = 1024
M = 1024
NCH = M // 128
NS = NB * NCH
NLM = float(NB * L * M)
MAGIC = 0x5F3759DF
# DVE Schraudolph exp: bf16 bits = int16(x * 128/ln2 + 16250.49); ~3% max rel
# err, exact dynamic range to e^+-88. Offloaded chunks relieve the ACT engine.
SCHRAU_A = 184.6635
SCHRAU_B = 16250.49
DVE_EXP_CHUNKS = (2, 4, 6)


def _newton_rsqrt(nc, x, y, t, magic_i32, iters=1):
    """y = 1/sqrt(x) entirely on DVE (bit-trick seed + Newton iters)."""
    xi = x.bitcast(I32)
    yi = y.bitcast(I32)
    nc.vector.tensor_scalar(
        out=yi, in0=xi, scalar1=1, scalar2=None, op0=OP.arith_shift_right
    )
    nc.vector.tensor_tensor(out=yi, in0=magic_i32, in1=yi, op=OP.subtract)
    for _ in range(iters):
        nc.vector.tensor_mul(t, y, y)
        nc.vector.tensor_mul(t, t, x)
        nc.vector.tensor_scalar(
            out=t, in0=t, scalar1=-0.5, scalar2=1.5, op0=OP.mult, op1=OP.add
        )
        nc.vector.tensor_mul(y, y, t)


def _strip_of(inst):
    """(c0, c1) column-strip a PE weight-load/matmul touches, 32-rounded."""
    pos = getattr(inst, "tile_position", None) or (0, 0)
    size = getattr(inst, "tile_size", None)
    if size is None:
        cols = 128
    else:
        cols = size[1]
    cols = 32 * ((cols + 31) // 32)
    return (pos[1], min(128, pos[1] + cols))


def _merge_sync(dst, src_inst):
    si = src_inst.sync_info
    if si is None:
        return
    nsi = dst.sync_info
    if nsi is None:
        dst.sync_info = si
        return
    if si.on_wait:
        nsi.on_wait.extend(si.on_wait)
    if si.on_update:
        nsi.on_update.extend(si.on_update)
    dst.sync_info = nsi


def _optimize_ldweights(nc):
    """Per-col-strip ldweights dedupe + hoisting for col-tiled concurrency.

    1. Remove an InstLdweights whose (AP, perf/transpose/position) signature
       is already resident in every 32-col strip it covers (j-pairs and the
       odd-batch den/vt reloads). Its waits merge into the next matmul.
    2. Hoist an InstLdweights above an immediately-preceding InstMatmult
       whose col strip is disjoint (so back-to-back matmuls into disjoint
       col groups issue without an intervening weight load and overlap).
    """
    removed = hoisted = 0
    for bb in nc.m.functions[0].blocks:
        insts = list(bb.instructions)
        out = []
        resident = {}  # strip c0 -> (c1, sig)

        def clear_overlap(c0, c1):
            for k in list(resident):
                k1 = resident[k][0]
                if k < c1 and c0 < k1:
                    del resident[k]

        k = 0
        while k < len(insts):
            inst = insts[k]
            tname = type(inst).__name__
            if tname == "InstLdweights":
                ap = inst.ins[0]
                sig = (
                    str(ap),
                    getattr(inst, "perf_mode", None),
                    getattr(inst, "is_transpose", None),
                    getattr(inst, "tile_position", None),
                )
                c0, c1 = _strip_of(inst)
                ent = resident.get(c0)
                nxt = insts[k + 1] if k + 1 < len(insts) else None
                if (
                    ent is not None
                    and ent == (c1, sig)
                    and nxt is not None
                    and type(nxt).__name__ == "InstMatmult"
                ):
                    _merge_sync(nxt, inst)
                    removed += 1
                    k += 1
                    continue
                clear_overlap(c0, c1)
                resident[c0] = (c1, sig)
                # hoist above preceding disjoint-strip matmuls
                j = len(out)
                while j > 0 and type(out[j - 1]).__name__ == "InstMatmult":
                    mc0, mc1 = _strip_of(out[j - 1])
                    if mc0 < c1 and c0 < mc1:
                        break
                    j -= 1
                if j < len(out):
                    out.insert(j, inst)
                    hoisted += 1
                else:
                    out.append(inst)
                k += 1
                continue
            elif tname == "InstMatmult":
                pass  # matmuls do not disturb the loaded stationary
            elif getattr(inst, "engine", None) == mybir.EngineType.PE and not getattr(
                inst, "is_sequencer_only", False
            ):
                resident.clear()
            out.append(inst)
            k += 1
        bb.instructions = out
    return removed, hoisted


def build_nc(debug: bool = False):
    nc = bacc.Bacc("TRN2", target_bir_lowering=False, debug=debug)

    q2_d = nc.dram_tensor("q2", [128, NB // 2, L], F16, kind="ExternalInput")
    k2_d = nc.dram_tensor("k2", [128, NB // 2, L], F16, kind="ExternalInput")
    qt_d = nc.dram_tensor("qt", [128, NB // 2, NCH, 128], F16, kind="ExternalInput")
    kt_d = nc.dram_tensor("kt", [128, NB // 2, NCH, 128], F16, kind="ExternalInput")
    vt_d = nc.dram_tensor("vt", [128, NB, NCH, 65], BF16, kind="ExternalInput")
    gsim_d = nc.dram_tensor("g_sim", [1, 1], F32, kind="ExternalInput")
    gval_d = nc.dram_tensor("gamma_val", [C, 1], F32, kind="ExternalInput")
    bval_d = nc.dram_tensor("beta_val", [C, 1], F32, kind="ExternalInput")
    out_d = nc.dram_tensor("out", [NB, C, L], F32, kind="ExternalOutput")

    with tile.TileContext(nc) as tc, ExitStack() as ctx:
        const_p = ctx.enter_context(tc.tile_pool(name="const", bufs=1))
        pt_p = ctx.enter_context(tc.tile_pool(name="pt", bufs=6))
        rec_p = ctx.enter_context(tc.tile_pool(name="rec", bufs=2))
        pout_p = ctx.enter_context(tc.tile_pool(name="pout", bufs=4))
        small = ctx.enter_context(tc.tile_pool(name="small", bufs=1))
        lg_p = ctx.enter_context(tc.tile_pool(name="lg", bufs=4, space="PSUM"))

        # ---- constants ----
        warm_w = const_p.tile([128, 32], F16, tag="warmw")
        nc.vector.memset(warm_w[:], 1.0)
        warm_m = const_p.tile([128, 512], F16, tag="warmm")
        nc.vector.memset(warm_m[:], 1.0)
        ones_col = const_p.tile([128, 1], F32, tag="ones_col")
        nc.vector.memset(ones_col[:], 1.0)
        ones_row = const_p.tile([1, 128], F32, tag="ones_row")
        nc.vector.memset(ones_row[:], 1.0)
        magic1 = const_p.tile([1, 1], I32, tag="magic1")
        nc.vector.memset(magic1[:], MAGIC)
        magicC = const_p.tile([C, 1], I32, tag="magicC")
        nc.vector.memset(magicC[:], MAGIC)
        tiny = const_p.tile([1, 2], F32, tag="tiny")
        nc.vector.memset(tiny[:], 1.0)
        ones_bf = const_p.tile([128, 1], BF16, tag="ones_bf")
        nc.vector.memset(ones_bf[:], 1.0)

        # ACT exp table preload (first table load costs ~1.3us; hide at t=0)
        nc.scalar.activation(tiny[:, 1:2], tiny[:, 0:1], AF.Exp)

        gsim_sb = const_p.tile([1, 1], F32, tag="gsim")
        gval_sb = const_p.tile([C, 1], F32, tag="gval")
        bval_sb = const_p.tile([C, 1], F32, tag="bval")

        qt_sb = const_p.tile([128, NB // 2, NCH, 128], F16, tag="qt")
        kt_sb = const_p.tile([128, NB // 2, NCH, 128], F16, tag="kt")
        q2_sb = const_p.tile([128, NB // 2, L], F16, tag="q2")
        k2_sb = const_p.tile([128, NB // 2, L], F16, tag="k2")
        vt_sb = const_p.tile([128, NB, NCH, 65], BF16, tag="vt")

        # input DMAs, priority order. sync/scalar are HWDGE; vt on SWDGE.
        for p in range(4):
            nc.sync.dma_start(out=qt_sb[:, p], in_=qt_d[:, p])
            nc.scalar.dma_start(out=kt_sb[:, p], in_=kt_d[:, p])
        nc.sync.dma_start(out=q2_sb[:, 0, :], in_=q2_d[:, 0, :])
        nc.scalar.dma_start(out=k2_sb[:, 0, :], in_=k2_d[:, 0, :])
        nc.sync.dma_start(out=gsim_sb[:], in_=gsim_d[:])
        nc.sync.dma_start(out=gval_sb[:], in_=gval_d[:])
        nc.sync.dma_start(out=bval_sb[:], in_=bval_d[:])
        for p in range(1, 4):
            nc.sync.dma_start(out=q2_sb[:, p, :], in_=q2_d[:, p, :])
            nc.scalar.dma_start(out=k2_sb[:, p, :], in_=k2_d[:, p, :])
        for b in range(NB):
            nc.gpsimd.dma_start(out=vt_sb[:, b], in_=vt_d[:, b])

        s_bcast = small.tile([128, 1], F32, tag="sbc")
        sa_bcast = small.tile([128, 1], F32, tag="sabc")
        # pair-packed ue: even batch of pair g on partitions 0-63; odd
        # batches drain into a 64-partition staging tile that an (idle)
        # DMA engine lifts to partitions 64-127 for pair-wide gelu+store.
        ue_sb = const_p.tile([128, NB // 2, 2, 512], F32, tag="ue")
        statsA = small.tile([C, 2, NB // 2, 2 * 6], F32, tag="statsA")

        lg_tiles = {}
        pt_tiles = {}
        pv_tiles = {}
        rec_tiles = {}

        def emit_qk(s):
            b, c = divmod(s, NCH)
            pair, r = divmod(b, 2)
            rs = slice(r * 64, r * 64 + 64)
            for j in range(2):
                lg = lg_p.tile([128, 512], F32, tag="lg", name="lg")
                nc.tensor.matmul(
                    lg[:],
                    k2_sb[rs, pair, c * 128 : (c + 1) * 128],
                    q2_sb[rs, pair, j * 512 : (j + 1) * 512],
                    start=True,
                    stop=True,
                )
                lg_tiles[(s, j)] = lg

        def emit_exp(s):
            c = s % NCH
            for j in range(2):
                lg = lg_tiles.pop((s, j))
                pt = pt_p.tile([128, 512], BF16, tag="pt", name="pt")
                if c in DVE_EXP_CHUNKS:
                    nc.vector.tensor_scalar(
                        out=pt.bitcast(I16)[:],
                        in0=lg[:],
                        scalar1=sa_bcast[:, 0:1],
                        scalar2=SCHRAU_B,
                        op0=OP.mult,
                        op1=OP.add,
                    )
                else:
                    nc.scalar.activation(pt[:], lg[:], AF.Exp, scale=s_bcast[:, 0:1])
                pt_tiles[(s, j)] = pt

        # ---- phase 0: warmup ramp + pair-gram variance -> s ----
        with tc.tile_pool(name="warm", bufs=1, space="PSUM") as warm_p, tc.tile_pool(
            name="gram", bufs=1, space="PSUM"
        ) as gram_p, tc.tile_pool(name="vs", bufs=1, space="PSUM") as vs_p:
            wps = warm_p.tile([32, 512], F32, tag="warm")
            for _ in range(6):
                nc.tensor.matmul(wps[:], warm_w[:], warm_m[:], start=True, stop=True)

            acc = small.tile([128, 4], F32, tag="acc")
            for p in range(4):
                gq_ps = gram_p.tile([128, 128], F32, tag="gq")
                gk_ps = gram_p.tile([128, 128], F32, tag="gk")
                for c in range(NCH):
                    nc.tensor.matmul(
                        gq_ps[:],
                        qt_sb[:, p, c],
                        qt_sb[:, p, c],
                        start=(c == 0),
                        stop=(c == NCH - 1),
                    )
                for c in range(NCH):
                    nc.tensor.matmul(
                        gk_ps[:],
                        kt_sb[:, p, c],
                        kt_sb[:, p, c],
                        start=(c == 0),
                        stop=(c == NCH - 1),
                    )
                gq_sb = small.tile([128, 128], F32, tag="gq_sb")
                nc.vector.tensor_copy(gq_sb[:], gq_ps[:])
                prod = small.tile([128, 128], F32, tag="prod")
                nc.vector.tensor_mul(prod[:], gq_sb[:], gk_ps[:])
                # only the block-diagonal quadrants are per-batch grams
                nc.vector.reduce_sum(
                    acc[0:64, p : p + 1], prod[0:64, 0:64], axis=mybir.AxisListType.X
                )
                nc.vector.reduce_sum(
                    acc[64:128, p : p + 1],
                    prod[64:128, 64:128],
                    axis=mybir.AxisListType.X,
                )

            # QK prefill while the s-chain resolves
            emit_qk(0)
            emit_qk(1)

            red = small.tile([128, 1], F32, tag="red")
            nc.vector.reduce_sum(red[:], acc[:], axis=mybir.AxisListType.X)
            vs_ps = vs_p.tile([128, 2], F32, tag="vs")
            var_ps = vs_ps[0:1, 0:1]
            sb_ps = vs_ps[:, 1:2]
            nc.tensor.matmul(var_ps, ones_col[:], red[:], start=True, stop=True)

            sv = small.tile([1, 6], F32, tag="sv")
            nc.vector.tensor_scalar(
                out=sv[:, 0:1],
                in0=var_ps,
                scalar1=1.0 / NLM,
                scalar2=EPS,
                op0=OP.mult,
                op1=OP.add,
            )
            _newton_rsqrt(nc, sv[:, 0:1], sv[:, 1:2], sv[:, 2:3], magic1[:])
            nc.vector.tensor_mul(sv[:, 3:4], sv[:, 1:2], gsim_sb[:])
            nc.tensor.matmul(sb_ps, ones_row[:], sv[:, 3:4], start=True, stop=True)
            nc.vector.tensor_copy(s_bcast[:], sb_ps)
            nc.vector.tensor_scalar_mul(sa_bcast[:], s_bcast[:], SCHRAU_A)

            # free-floating bridge matmuls (scheduler places them)
            for _ in range(6):
                nc.tensor.matmul(
                    wps[:], warm_w[:], warm_m[:], start=True, stop=True
                )

        emit_exp(0)

        # ---- phase A: software-pipelined [QK(s+2), PV(s)] slots ----
        den_tiles = {}
        stage_tiles = {}

        def drain_copy(b, lo, hi):
            if b not in den_tiles:
                den_tiles[b] = rec_p.tile([1, L], F32, tag="den0", name="den0")
            nc.vector.tensor_copy(den_tiles[b][:, lo:hi], pv_tiles[b][64:65, lo:hi])

        def drain_recip(b, lo, hi):
            nc.vector.reciprocal_approx_fast(
                out=den_tiles[b][:, lo:hi], in_=den_tiles[b][:, lo:hi]
            )

        def drain_bcast(b, lo, hi):
            if b not in rec_tiles:
                rec_tiles[b] = rec_p.tile([C, L], F32, tag="rec", name="rec")
            nc.gpsimd.partition_broadcast(
                rec_tiles[b][:, lo:hi], den_tiles[b][:, lo:hi]
            )

        def drain_mul(b, lo, hi):
            g = b // 2
            hs = slice(lo // 512, (hi + 511) // 512)
            if b % 2 == 0:
                out = ue_sb[0:64, g, hs, :]
            else:
                if b not in stage_tiles:
                    stage_tiles[b] = rec_p.tile(
                        [C, 2, 512], F32, tag="stage", name="stage"
                    )
                out = stage_tiles[b][:, hs, :]
            nc.vector.tensor_mul(
                out, pv_tiles[b][0:C, lo:hi], rec_tiles[b][:, lo:hi]
            )

        def drain_lift(b):
            # idle DMA engines lift the odd batch to partitions 64-127
            g = b // 2
            nc.sync.dma_start(out=ue_sb[64:128, g], in_=stage_tiles[b][:])

        def drain_stats(b, h):
            g, par = divmod(b, 2)
            srcap = (
                ue_sb[0:64, g, h, :] if par == 0 else stage_tiles[b][:, h, :]
            )
            nc.vector.bn_stats(statsA[:, par, g, h * 6 : (h + 1) * 6], srcap)

        with tc.tile_pool(name="pv", bufs=2, space="PSUM") as pv_p:
            for s in range(NS):
                b, c = divmod(s, NCH)
                if s + 1 < NS:
                    emit_exp(s + 1)
                if c == 0:
                    pv_tiles[b] = pv_p.tile([128, L], F32, tag="pv", name="pv")
                if s + 2 < NS:
                    emit_qk(s + 2)
                pv = pv_tiles[b]
                for j in range(2):
                    pt = pt_tiles.pop((s, j))
                    nc.tensor.matmul(
                        pv[0:65, j * 512 : (j + 1) * 512],
                        vt_sb[:, b, c, :],
                        pt[:],
                        start=(c == 0),
                        stop=(c == NCH - 1),
                        skip_group_check=True,
                    )
                if b >= 1:
                    bb = b - 1
                    if c == 1:
                        drain_copy(bb, 0, L)
                        drain_recip(bb, 0, L)
                    elif c == 2:
                        drain_bcast(bb, 0, L)
                    elif c == 3:
                        drain_mul(bb, 0, L)
                    elif c == 4:
                        drain_stats(bb, 0)
                        if bb % 2 == 1:
                            drain_lift(bb)
                    elif c == 5:
                        drain_stats(bb, 1)
                        pv_tiles.pop(bb)
                        rec_tiles.pop(bb)
                        den_tiles.pop(bb)

            # last batch drains in pipelined quarters (DVE chain) with
            # the gpsimd broadcasts and stats trailing per half
            for q in range(4):
                drain_copy(NB - 1, q * 256, (q + 1) * 256)
                drain_recip(NB - 1, q * 256, (q + 1) * 256)
                if q % 2 == 1:
                    drain_bcast(NB - 1, (q - 1) * 256, (q + 1) * 256)
                    drain_mul(NB - 1, (q - 1) * 256, (q + 1) * 256)
                    drain_stats(NB - 1, q // 2)
            drain_lift(NB - 1)
            pv_tiles.pop(NB - 1)
            rec_tiles.pop(NB - 1)
            den_tiles.pop(NB - 1)

        # ---- phase B: val-BN affine (64-wide), ab lifted to 128, pair-wide
        # gelu + full-fanout pair stores ----
        chan = small.tile([C, 2], F32, tag="chan")
        nc.vector.bn_aggr(chan[:], statsA[:])
        # gelu ACT table preload: dep on chan keeps it post-drains; the
        # 1.5us table load overlaps the affine chain below
        nc.scalar.activation(tiny[:, 1:2], chan[0:1, 0:1], AF.Gelu)
        vb = small.tile([C, 6], F32, tag="vb")
        nc.vector.tensor_scalar_add(vb[:, 0:1], chan[:, 1:2], EPS)
        _newton_rsqrt(nc, vb[:, 0:1], vb[:, 1:2], vb[:, 2:3], magicC[:])
        ab = small.tile([128, 2], F32, tag="ab")
        nc.vector.tensor_mul(ab[0:64, 0:1], gval_sb[:], vb[:, 1:2])
        nc.vector.tensor_mul(vb[:, 3:4], chan[:, 0:1], ab[0:64, 0:1])
        nc.vector.tensor_sub(ab[0:64, 1:2], bval_sb[:], vb[:, 3:4])
        nc.vector.stream_shuffle(ab[64:128, :], ab[0:64, :], mask=list(range(32)))

        for g in range(4):
            pout = pout_p.tile([128, 2, 512], F32, tag="pout")
            nc.scalar.activation(
                pout[:],
                ue_sb[:, g],
                AF.Gelu,
                scale=ab[:, 0:1],
                bias=ab[:, 1:2],
            )
            eng = nc.sync if g % 2 == 0 else nc.scalar
            eng.dma_start(out=out_d[2 * g : 2 * g + 2], in_=pout[:])

    n, h = _optimize_ldweights(nc)
    print(f"deduped {n} ldweights, hoisted {h}", file=sys.stderr)
    nc.compile()
    return nc


def make_in_map(q, k, v, gamma_sim, beta_sim, gamma_val, beta_val, h):
    """Build the per-core (per-head) input map. Layout-only host prep."""
    import ml_dtypes

    qh = q[:, h]
    kh = k[:, h]
    vh = v[:, h]

    def two(x):
        # [p = r*64+d, pair, l] <- x[2*pair+r, d, l]
        return np.ascontiguousarray(
            x.reshape(4, 2, 64, L).transpose(1, 2, 0, 3).reshape(128, 4, L)
        ).astype(np.float16)

    def gramt(x):
        # [lp, pair, c, j=r*64+d] <- x[2*pair+r, d, c*128+lp]
        t = x.reshape(4, 2, 64, NCH, 128).transpose(4, 0, 3, 1, 2)
        return np.ascontiguousarray(t.reshape(128, 4, NCH, 128)).astype(np.float16)

    def vt5(x):
        # [mp, b, c, j]. even b: v in cols 0-63, ones col 64 (rv on PSUM
        # rows 0-63, den row 64). odd b: ones col 32 (32-aligned: walrus
        # rejects partition base 63), v in cols 64-127 (rv rows 64-127,
        # den row 32). Pair-packs a batch pair onto all 128 partitions.
        out = np.ones((128, NB, NCH, 65), np.float32)
        out[:, :, :, 0:64] = x.reshape(NB, 64, NCH, 128).transpose(3, 0, 2, 1)
        return out.astype(ml_dtypes.bfloat16)

    return {
        "q2": two(qh),
        "k2": two(kh),
        "qt": gramt(qh),
        "kt": gramt(kh),
        "vt": vt5(vh),
        "g_sim": np.asarray(gamma_sim[h], dtype=np.float32).reshape(1, 1),
        "gamma_val": np.asarray(
            gamma_val[h * C : (h + 1) * C], dtype=np.float32
        ).reshape(C, 1),
        "beta_val": np.asarray(
            beta_val[h * C : (h + 1) * C], dtype=np.float32
        ).reshape(C, 1),
    }


_CACHED_NC = None


def _setup_profiling():
    """Make run_bass_kernel_spmd(trace=True) work on images missing
    antenv.axon_hooks: inject the ctypes NTFF hook + keep artifacts local."""
    import contextlib
    import ctypes
    import types

    try:
        from antenv.axon_hooks import get_axon_ntff_profile_hook  # noqa: F401
    except ImportError:
        so_path = os.environ.get("AXON_PJRT_SO", "/opt/axon/libaxon_pjrt.so")
        lib = ctypes.CDLL(so_path)
        lib.axon_start_nrt_profile.argtypes = [
            ctypes.POINTER(ctypes.c_int64),
            ctypes.c_size_t,
        ]
        lib.axon_start_nrt_profile.restype = ctypes.c_int64
        lib.axon_stop_nrt_profile.argtypes = [ctypes.c_char_p]
        lib.axon_stop_nrt_profile.restype = ctypes.c_int64

        @contextlib.contextmanager
        def _hook(output_dir, device_ids):
            import jax

            jax.devices()
            if device_ids:
                ids = (ctypes.c_int64 * len(device_ids))(*device_ids)
                rc = lib.axon_start_nrt_profile(ids, len(device_ids))
            else:
                rc = lib.axon_start_nrt_profile(None, 0)
            if rc != 0:
                raise RuntimeError(f"axon_start_nrt_profile rc={rc}")
            try:
                yield
            finally:
                n = lib.axon_stop_nrt_profile(str(output_dir).encode())
                print(f"ntff profile: {n} file(s) -> {output_dir}", file=sys.stderr)

        mod = types.ModuleType("antenv.axon_hooks")
        mod.get_axon_ntff_profile_hook = lambda: _hook
        mod.set_axon_ntff_profile_hook = lambda h: None
        import antenv

        sys.modules["antenv.axon_hooks"] = mod
        antenv.axon_hooks = mod

    import concourse.bass_utils as bu

    bu.upload_artifacts = lambda tmpdir: f"local://{tmpdir}"


def kernel(query, key, value, gamma_sim, beta_sim, gamma_val, beta_val):
    global _CACHED_NC
    from concourse.bass_utils import run_bass_kernel_spmd

    query = np.asarray(query, dtype=np.float32)
    key = np.asarray(key, dtype=np.float32)
    value = np.asarray(value, dtype=np.float32)
    gamma_sim = np.asarray(gamma_sim, dtype=np.float32)
    gamma_val = np.asarray(gamma_val, dtype=np.float32)
    beta_val = np.asarray(beta_val, dtype=np.float32)

    if _CACHED_NC is None:
        _CACHED_NC = build_nc()
    nc = _CACHED_NC

    in_maps = [
        make_in_map(query, key, value, gamma_sim, None, gamma_val, beta_val, h)
        for h in range(8)
    ]
    trace = bool(int(os.environ.get("BASS_PROFILE", "0")))
    tmpdir = os.environ.get("BASS_PROFILE_DIR") or None
    if trace:
        try:
            _setup_profiling()
        except Exception as e:  # noqa: BLE001
            print(f"profiling setup failed ({e}); running untraced", file=sys.stderr)
            trace = False
    try:
        res = run_bass_kernel_spmd(
            nc, in_maps, list(range(8)), trace=trace, tmpdir=tmpdir
        )
    except Exception:
        if not trace:
            raise
        print("traced run failed; retrying untraced", file=sys.stderr)
        res = run_bass_kernel_spmd(nc, in_maps, list(range(8)), trace=False)
    if trace and res.exec_time_ns is not None:
        print(f"HW exec time: {res.exec_time_ns} ns")

    out = np.empty((NB, 8 * C, L), dtype=np.float32)
    for h in range(8):
        out[:, h * C : (h + 1) * C, :] = res.results[h]["out"]
    return out


# revision 25
# speedup vs baseline: 1.3674x; 1.3674x over previous
"""Trainium2 Bass kernel for nn_AttentionOperation_32521492365427.

kernel(**inputs) -> np.ndarray, full shapes:
  query/key/value: [8, 8, 64, 1024] f32; gamma_sim/beta_sim: [8];
  gamma_val/beta_val: [512]; output: [8, 512, 1024] f32.

Sharded by HEAD across the 8 NeuronCores (one head per core): both
BatchNorms then have core-local statistics, so there are no collectives.

Per-core math:
 - softmax is shift-invariant => the sim-BN reduces to one per-head scale
   s = gamma_sim / sqrt(var(logits) + EPS); beta/mean drop out (the
   mean^2 term of the variance is O(1e-8) of E[l^2] and is dropped).
 - sumsq(logits_b) = <Gq_b, Gk_b> over 64x64 Gram matrices; the grams of
   batch pairs (2p, 2p+1) are block-diagonal-packed into [128,128]
   matmuls (half the instructions of per-batch grams).
 - PV stationary is [128_m, 64 v | 1 ones]: PV row 64 is the softmax
   denominator; normalize = DVE copy+reciprocal of the den row, gpsimd
   partition-broadcast, DVE multiply straight out of PSUM.
 - val-BN affine + exact (erf) gelu fuse into one ACT pass per batch
   pair; s is broadcast to 128 partitions via a tiny ones-matmul.

Scheduling:
 - PE stream: warmup ramp -> pair-grams -> QK prefill -> bridge matmuls
   over the s-chain latency -> software-pipelined [QK(s+2), PV(s)]
   slots. The tensor engine clock (HAM) needs ~3us of continuous
   activity for 2.4 GHz; idle gaps halve it to 1.2 GHz.
 - logits/pt live as [128,512] half-tiles (lg pool bufs=4 = 4 PSUM
   banks, pv double-buffered = 4 banks): the exp->QK write-after-read
   chain is half-granular so it never serializes the slot pipeline.
 - exp: ACT table exp for chunks {0,1,3,5,7}, DVE Schraudolph bf16-bits
   exp for {2,4,6} (logits reach +-43 sigma, so bf16 range is required).
 - per-batch drains (copy/recip/bcast/mul/bn_stats) pipeline inside the
   next batch's slots; odd batches drain to a staging tile that an idle
   DMA engine lifts to partitions 64-127, so gelu + output stores run
   128 partitions wide (full SDMA fanout) per batch pair. The last
   batch drains in quarter-granular pipelined steps.
 - gelu ACT table preloads via a dummy op gated on the BN aggregate;
   output DMAs alternate between the SP and ACT HWDGE queues.
"""

import os
import sys

sys.path.insert(0, "/opt/trn_rl_repo")

from contextlib import ExitStack

import numpy as np

import concourse.bacc as bacc
import concourse.bass as bass  # noqa: F401
import concourse.tile as tile
from concourse import mybir

F32 = mybir.dt.float32
F16 = mybir.dt.float16
BF16 = mybir.dt.bfloat16
I32 = mybir.dt.int32
I16 = mybir.dt.int16
AF = mybir.ActivationFunctionType
OP = mybir.AluOpType

EPS = 1e-3
NB = 8
D = 64
C = 64
L = 1024
M = 1024
NCH = M // 128
NS = NB * NCH
NLM = float(NB * L * M)
MAGIC = 0x5F3759DF
# DVE Schraudolph exp: bf16 bits = int16(x * 128/ln2 + 16250.49); ~3% max rel
# err, exact dynamic range to e^+-88. Offloaded chunks relieve the ACT engine.
SCHRAU_A = 184.6635
SCHRAU_B = 16250.49
DVE_EXP_CHUNKS = (2, 4, 6)


def _newton_rsqrt(nc, x, y, t, magic_i32, iters=1):
    """y = 1/sqrt(x) entirely on DVE (bit-trick seed + Newton iters)."""
    xi = x.bitcast(I32)
    yi = y.bitcast(I32)
    nc.vector.tensor_scalar(
        out=yi, in0=xi, scalar1=1, scalar2=None, op0=OP.arith_shift_right
    )
    nc.vector.tensor_tensor(out=yi, in0=magic_i32, in1=yi, op=OP.subtract)
    for _ in range(iters):
        nc.vector.tensor_mul(t, y, y)
        nc.vector.tensor_mul(t, t, x)
        nc.vector.tensor_scalar(
            out=t, in0=t, scalar1=-0.5, scalar2=1.5, op0=OP.mult, op1=OP.add
        )
        nc.vector.tensor_mul(y, y, t)


def _strip_of(inst):
    """(c0, c1) column-strip a PE weight-load/matmul touches, 32-rounded."""
    pos = getattr(inst, "tile_position", None) or (0, 0)
    size = getattr(inst, "tile_size", None)
    if size is None:
        cols = 128
    else:
        cols = size[1]
    cols = 32 * ((cols + 31) // 32)
    return (pos[1], min(128, pos[1] + cols))


def _merge_sync(dst, src_inst):
    si = src_inst.sync_info
    if si is None:
        return
    nsi = dst.sync_info
    if nsi is None:
        dst.sync_info = si
        return
    if si.on_wait:
        nsi.on_wait.extend(si.on_wait)
    if si.on_update:
        nsi.on_update.extend(si.on_update)
    dst.sync_info = nsi


def _optimize_ldweights(nc):
    """Per-col-strip ldweights dedupe + hoisting for col-tiled concurrency.

    1. Remove an InstLdweights whose (AP, perf/transpose/position) signature
       is already resident in every 32-col strip it covers (j-pairs and the
       odd-batch den/vt reloads). Its waits merge into the next matmul.
    2. Hoist an InstLdweights above an immediately-preceding InstMatmult
       whose col strip is disjoint (so back-to-back matmuls into disjoint
       col groups issue without an intervening weight load and overlap).
    """
    removed = hoisted = 0
    for bb in nc.m.functions[0].blocks:
        insts = list(bb.instructions)
        out = []
        resident = {}  # strip c0 -> (c1, sig)

        def clear_overlap(c0, c1):
            for k in list(resident):
                k1 = resident[k][0]
                if k < c1 and c0 < k1:
                    del resident[k]

        k = 0
        while k < len(insts):
            inst = insts[k]
            tname = type(inst).__name__
            if tname == "InstLdweights":
                ap = inst.ins[0]
                sig = (
                    str(ap),
                    getattr(inst, "perf_mode", None),
                    getattr(inst, "is_transpose", None),
                    getattr(inst, "tile_position", None),
                )
                c0, c1 = _strip_of(inst)
                ent = resident.get(c0)
                nxt = insts[k + 1] if k + 1 < len(insts) else None
                if (
                    ent is not None
                    and ent == (c1, sig)
                    and nxt is not None
                    and type(nxt).__name__ == "InstMatmult"
                ):
                    _merge_sync(nxt, inst)
                    removed += 1
                    k += 1
                    continue
                clear_overlap(c0, c1)
                resident[c0] = (c1, sig)
                # hoist above preceding disjoint-strip matmuls
                j = len(out)
                while j > 0 and type(out[j - 1]).__name__ == "InstMatmult":
                    mc0, mc1 = _strip_of(out[j - 1])
                    if mc0 < c1 and c0 < mc1:
                        break
                    j -= 1
                if j < len(out):
                    out.insert(j, inst)
                    hoisted += 1
                else:
                    out.append(inst)
                k += 1
                continue
            elif tname == "InstMatmult":
                pass  # matmuls do not disturb the loaded stationary
            elif getattr(inst, "engine", None) == mybir.EngineType.PE and not getattr(
                inst, "is_sequencer_only", False
            ):
                resident.clear()
            out.append(inst)
            k += 1
        bb.instructions = out
    return removed, hoisted


def build_nc(debug: bool = False):
    nc = bacc.Bacc("TRN2", target_bir_lowering=False, debug=debug)

    q2_d = nc.dram_tensor("q2", [128, NB, L], F16, kind="ExternalInput")
    k2_d = nc.dram_tensor("k2", [128, NB, L], F16, kind="ExternalInput")
    qt_d = nc.dram_tensor("qt", [128, NB // 2, NCH, 128], F16, kind="ExternalInput")
    kt_d = nc.dram_tensor("kt", [128, NB // 2, NCH, 128], F16, kind="ExternalInput")
    vt_d = nc.dram_tensor("vt", [128, NB, NCH, 65], BF16, kind="ExternalInput")
    gsim_d = nc.dram_tensor("g_sim", [1, 1], F32, kind="ExternalInput")
    gval_d = nc.dram_tensor("gamma_val", [C, 1], F32, kind="ExternalInput")
    bval_d = nc.dram_tensor("beta_val", [C, 1], F32, kind="ExternalInput")
    out_d = nc.dram_tensor("out", [NB, C, L], F32, kind="ExternalOutput")

    with tile.TileContext(nc) as tc, ExitStack() as ctx:
        const_p = ctx.enter_context(tc.tile_pool(name="const", bufs=1))
        pt_p = ctx.enter_context(tc.tile_pool(name="pt", bufs=6))
        rec_p = ctx.enter_context(tc.tile_pool(name="rec", bufs=2))
        pout_p = ctx.enter_context(tc.tile_pool(name="pout", bufs=4))
        small = ctx.enter_context(tc.tile_pool(name="small", bufs=1))
        lg_p = ctx.enter_context(tc.tile_pool(name="lg", bufs=4, space="PSUM"))

        # ---- constants ----
        warm_w = const_p.tile([128, 32], F16, tag="warmw")
        nc.vector.memset(warm_w[:], 1.0)
        warm_m = const_p.tile([128, 512], F16, tag="warmm")
        nc.vector.memset(warm_m[:], 1.0)
        ones_col = const_p.tile([128, 1], F32, tag="ones_col")
        nc.vector.memset(ones_col[:], 1.0)
        ones_row = const_p.tile([1, 128], F32, tag="ones_row")
        nc.vector.memset(ones_row[:], 1.0)
        magic1 = const_p.tile([1, 1], I32, tag="magic1")
        nc.vector.memset(magic1[:], MAGIC)
        magicC = const_p.tile([C, 1], I32, tag="magicC")
        nc.vector.memset(magicC[:], MAGIC)
        tiny = const_p.tile([1, 2], F32, tag="tiny")
        nc.vector.memset(tiny[:], 1.0)
        ones_bf = const_p.tile([128, 1], BF16, tag="ones_bf")
        nc.vector.memset(ones_bf[:], 1.0)

        # ACT exp table preload (first table load costs ~1.3us; hide at t=0)
        nc.scalar.activation(tiny[:, 1:2], tiny[:, 0:1], AF.Exp)

        gsim_sb = const_p.tile([1, 1], F32, tag="gsim")
        gval_sb = const_p.tile([C, 1], F32, tag="gval")
        bval_sb = const_p.tile([C, 1], F32, tag="bval")

        qt_sb = const_p.tile([128, NB // 2, NCH, 128], F16, tag="qt")
        kt_sb = const_p.tile([128, NB // 2, NCH, 128], F16, tag="kt")
        q2_sb = const_p.tile([128, NB, L], F16, tag="q2")
        k2_sb = const_p.tile([128, NB, L], F16, tag="k2")
        vt_sb = const_p.tile([128, NB, NCH, 65], BF16, tag="vt")

        # input DMAs, priority order. sync/scalar are HWDGE; vt on SWDGE.
        for p in range(4):
            nc.sync.dma_start(out=qt_sb[:, p], in_=qt_d[:, p])
            nc.scalar.dma_start(out=kt_sb[:, p], in_=kt_d[:, p])
        nc.sync.dma_start(out=q2_sb[:, 0:2], in_=q2_d[:, 0:2])
        nc.scalar.dma_start(out=k2_sb[:, 0:2], in_=k2_d[:, 0:2])
        nc.sync.dma_start(out=gsim_sb[:], in_=gsim_d[:])
        nc.sync.dma_start(out=gval_sb[:], in_=gval_d[:])
        nc.sync.dma_start(out=bval_sb[:], in_=bval_d[:])
        for g in range(1, 4):
            nc.sync.dma_start(out=q2_sb[:, 2 * g : 2 * g + 2], in_=q2_d[:, 2 * g : 2 * g + 2])
            nc.scalar.dma_start(out=k2_sb[:, 2 * g : 2 * g + 2], in_=k2_d[:, 2 * g : 2 * g + 2])
        for b in range(NB):
            nc.gpsimd.dma_start(out=vt_sb[:, b], in_=vt_d[:, b])

        s_bcast = small.tile([128, 1], F32, tag="sbc")
        sa_bcast = small.tile([128, 1], F32, tag="sabc")
        # pair-packed ue: even batch of pair g on partitions 0-63; odd
        # batches drain into a 64-partition staging tile that an (idle)
        # DMA engine lifts to partitions 64-127 for pair-wide gelu+store.
        ue_sb = const_p.tile([128, NB // 2, 2, 512], F32, tag="ue")
        statsA = small.tile([C, 2, NB // 2, 2 * 6], F32, tag="statsA")

        lg_tiles = {}
        pt_tiles = {}
        pv_tiles = {}
        rec_tiles = {}

        def emit_qk(s):
            # q/k are host-duplicated on both partition halves: the j0 half
            # contracts array rows 0-63 and j1 rows 64-127 (disjoint row
            # groups), so the two 512-col matmuls overlap in the PE array.
            b, c = divmod(s, NCH)
            for j in range(2):
                rs = slice(64 * j, 64 * j + 64)
                lg = lg_p.tile([128, 512], F32, tag="lg", name="lg")
                nc.tensor.matmul(
                    lg[:],
                    k2_sb[rs, b, c * 128 : (c + 1) * 128],
                    q2_sb[rs, b, j * 512 : (j + 1) * 512],
                    start=True,
                    stop=True,
                )
                lg_tiles[(s, j)] = lg

        def emit_exp(s):
            c = s % NCH
            for j in range(2):
                lg = lg_tiles.pop((s, j))
                pt = pt_p.tile([128, 512], BF16, tag="pt", name="pt")
                if c in DVE_EXP_CHUNKS:
                    nc.vector.tensor_scalar(
                        out=pt.bitcast(I16)[:],
                        in0=lg[:],
                        scalar1=sa_bcast[:, 0:1],
                        scalar2=SCHRAU_B,
                        op0=OP.mult,
                        op1=OP.add,
                    )
                else:
                    nc.scalar.activation(pt[:], lg[:], AF.Exp, scale=s_bcast[:, 0:1])
                pt_tiles[(s, j)] = pt

        # ---- phase 0: warmup ramp + pair-gram variance -> s ----
        with tc.tile_pool(name="warm", bufs=1, space="PSUM") as warm_p, tc.tile_pool(
            name="gram", bufs=1, space="PSUM"
        ) as gram_p, tc.tile_pool(name="vs", bufs=1, space="PSUM") as vs_p:
            wps = warm_p.tile([32, 512], F32, tag="warm")
            for _ in range(6):
                nc.tensor.matmul(wps[:], warm_w[:], warm_m[:], start=True, stop=True)

            acc = small.tile([128, 4], F32, tag="acc")
            for p in range(4):
                gq_ps = gram_p.tile([128, 128], F32, tag="gq")
                gk_ps = gram_p.tile([128, 128], F32, tag="gk")
                for c in range(NCH):
                    nc.tensor.matmul(
                        gq_ps[:],
                        qt_sb[:, p, c],
                        qt_sb[:, p, c],
                        start=(c == 0),
                        stop=(c == NCH - 1),
                    )
                for c in range(NCH):
                    nc.tensor.matmul(
                        gk_ps[:],
                        kt_sb[:, p, c],
                        kt_sb[:, p, c],
                        start=(c == 0),
                        stop=(c == NCH - 1),
                    )
                gq_sb = small.tile([128, 128], F32, tag="gq_sb")
                nc.vector.tensor_copy(gq_sb[:], gq_ps[:])
                prod = small.tile([128, 128], F32, tag="prod")
                nc.vector.tensor_mul(prod[:], gq_sb[:], gk_ps[:])
                # only the block-diagonal quadrants are per-batch grams
                nc.vector.reduce_sum(
                    acc[0:64, p : p + 1], prod[0:64, 0:64], axis=mybir.AxisListType.X
                )
                nc.vector.reduce_sum(
                    acc[64:128, p : p + 1],
                    prod[64:128, 64:128],
                    axis=mybir.AxisListType.X,
                )

            # QK prefill while the s-chain resolves
            emit_qk(0)
            emit_qk(1)

            red = small.tile([128, 1], F32, tag="red")
            nc.vector.reduce_sum(red[:], acc[:], axis=mybir.AxisListType.X)
            vs_ps = vs_p.tile([128, 2], F32, tag="vs")
            var_ps = vs_ps[0:1, 0:1]
            sb_ps = vs_ps[:, 1:2]
            nc.tensor.matmul(var_ps, ones_col[:], red[:], start=True, stop=True)

            sv = small.tile([1, 6], F32, tag="sv")
            nc.vector.tensor_scalar(
                out=sv[:, 0:1],
                in0=var_ps,
                scalar1=1.0 / NLM,
                scalar2=EPS,
                op0=OP.mult,
                op1=OP.add,
            )
            _newton_rsqrt(nc, sv[:, 0:1], sv[:, 1:2], sv[:, 2:3], magic1[:])
            nc.vector.tensor_mul(sv[:, 3:4], sv[:, 1:2], gsim_sb[:])
            nc.tensor.matmul(sb_ps, ones_row[:], sv[:, 3:4], start=True, stop=True)
            nc.vector.tensor_copy(s_bcast[:], sb_ps)
            nc.vector.tensor_scalar_mul(sa_bcast[:], s_bcast[:], SCHRAU_A)

            # free-floating bridge matmuls (scheduler places them)
            for _ in range(6):
                nc.tensor.matmul(
                    wps[:], warm_w[:], warm_m[:], start=True, stop=True
                )

        emit_exp(0)

        # ---- phase A: software-pipelined [QK(s+2), PV(s)] slots ----
        den_tiles = {}
        stage_tiles = {}

        def drain_copy(b, lo, hi):
            if b not in den_tiles:
                den_tiles[b] = rec_p.tile([1, L], F32, tag="den0", name="den0")
            nc.vector.tensor_copy(den_tiles[b][:, lo:hi], pv_tiles[b][64:65, lo:hi])

        def drain_recip(b, lo, hi):
            nc.vector.reciprocal_approx_fast(
                out=den_tiles[b][:, lo:hi], in_=den_tiles[b][:, lo:hi]
            )

        def drain_bcast(b, lo, hi):
            if b not in rec_tiles:
                rec_tiles[b] = rec_p.tile([C, L], F32, tag="rec", name="rec")
            nc.gpsimd.partition_broadcast(
                rec_tiles[b][:, lo:hi], den_tiles[b][:, lo:hi]
            )

        def drain_mul(b, lo, hi):
            g = b // 2
            hs = slice(lo // 512, (hi + 511) // 512)
            if b % 2 == 0:
                out = ue_sb[0:64, g, hs, :]
            else:
                if b not in stage_tiles:
                    stage_tiles[b] = rec_p.tile(
                        [C, 2, 512], F32, tag="stage", name="stage"
                    )
                out = stage_tiles[b][:, hs, :]
            nc.vector.tensor_mul(
                out, pv_tiles[b][0:C, lo:hi], rec_tiles[b][:, lo:hi]
            )

        def drain_lift(b):
            # idle DMA engines lift the odd batch to partitions 64-127
            g = b // 2
            nc.sync.dma_start(out=ue_sb[64:128, g], in_=stage_tiles[b][:])

        def drain_stats(b, h):
            g, par = divmod(b, 2)
            srcap = (
                ue_sb[0:64, g, h, :] if par == 0 else stage_tiles[b][:, h, :]
            )
            nc.vector.bn_stats(statsA[:, par, g, h * 6 : (h + 1) * 6], srcap)

        with tc.tile_pool(name="pv", bufs=2, space="PSUM") as pv_p:
            for s in range(NS):
                b, c = divmod(s, NCH)
                if s + 1 < NS:
                    emit_exp(s + 1)
                if c == 0:
                    pv_tiles[b] = pv_p.tile([128, L], F32, tag="pv", name="pv")
                if s + 2 < NS:
                    emit_qk(s + 2)
                pv = pv_tiles[b]
                for j in range(2):
                    pt = pt_tiles.pop((s, j))
                    nc.tensor.matmul(
                        pv[0:65, j * 512 : (j + 1) * 512],
                        vt_sb[:, b, c, :],
                        pt[:],
                        start=(c == 0),
                        stop=(c == NCH - 1),
                        skip_group_check=True,
                    )
                if b >= 1:
                    bb = b - 1
                    if c == 1:
                        drain_copy(bb, 0, L)
                        drain_recip(bb, 0, L)
                    elif c == 2:
                        drain_bcast(bb, 0, L)
                    elif c == 3:
                        drain_mul(bb, 0, L)
                    elif c == 4:
                        drain_stats(bb, 0)
                        if bb % 2 == 1:
                            drain_lift(bb)
                    elif c == 5:
                        drain_stats(bb, 1)
                        pv_tiles.pop(bb)
                        rec_tiles.pop(bb)
                        den_tiles.pop(bb)

            # last batch drains in pipelined quarters (DVE chain) with
            # the gpsimd broadcasts and stats trailing per half
            for q in range(4):
                drain_copy(NB - 1, q * 256, (q + 1) * 256)
                drain_recip(NB - 1, q * 256, (q + 1) * 256)
                if q % 2 == 1:
                    drain_bcast(NB - 1, (q - 1) * 256, (q + 1) * 256)
                    drain_mul(NB - 1, (q - 1) * 256, (q + 1) * 256)
                    drain_stats(NB - 1, q // 2)
            drain_lift(NB - 1)
            pv_tiles.pop(NB - 1)
            rec_tiles.pop(NB - 1)
            den_tiles.pop(NB - 1)

        # ---- phase B: val-BN affine (64-wide), ab lifted to 128, pair-wide
        # gelu + full-fanout pair stores ----
        chan = small.tile([C, 2], F32, tag="chan")
        nc.vector.bn_aggr(chan[:], statsA[:])
        # gelu ACT table preload: dep on chan keeps it post-drains; the
        # 1.5us table load overlaps the affine chain below
        nc.scalar.activation(tiny[:, 1:2], chan[0:1, 0:1], AF.Gelu)
        vb = small.tile([C, 6], F32, tag="vb")
        nc.vector.tensor_scalar_add(vb[:, 0:1], chan[:, 1:2], EPS)
        _newton_rsqrt(nc, vb[:, 0:1], vb[:, 1:2], vb[:, 2:3], magicC[:])
        ab = small.tile([128, 2], F32, tag="ab")
        nc.vector.tensor_mul(ab[0:64, 0:1], gval_sb[:], vb[:, 1:2])
        nc.vector.tensor_mul(vb[:, 3:4], chan[:, 0:1], ab[0:64, 0:1])
        nc.vector.tensor_sub(ab[0:64, 1:2], bval_sb[:], vb[:, 3:4])
        nc.vector.stream_shuffle(ab[64:128, :], ab[0:64, :], mask=list(range(32)))

        for g in range(4):
            pout = pout_p.tile([128, 2, 512], F32, tag="pout")
            nc.scalar.activation(
                pout[:],
                ue_sb[:, g],
                AF.Gelu,
                scale=ab[:, 0:1],
                bias=ab[:, 1:2],
            )
            eng = nc.sync if g % 2 == 0 else nc.scalar
            eng.dma_start(out=out_d[2 * g : 2 * g + 2], in_=pout[:])

    n, h = _optimize_ldweights(nc)
    print(f"deduped {n} ldweights, hoisted {h}", file=sys.stderr)
    nc.compile()
    return nc


def make_in_map(q, k, v, gamma_sim, beta_sim, gamma_val, beta_val, h):
    """Build the per-core (per-head) input map. Layout-only host prep."""
    import ml_dtypes

    qh = q[:, h]
    kh = k[:, h]
    vh = v[:, h]

    def two(x):
        # [p, b, l] <- x[b, p % 64, l]: each batch duplicated on both
        # partition halves so QK j-halves can use disjoint PE row groups
        t = x.transpose(1, 0, 2)
        return np.ascontiguousarray(
            np.concatenate([t, t], axis=0)
        ).astype(np.float16)

    def gramt(x):
        # [lp, pair, c, j=r*64+d] <- x[2*pair+r, d, c*128+lp]
        t = x.reshape(4, 2, 64, NCH, 128).transpose(4, 0, 3, 1, 2)
        return np.ascontiguousarray(t.reshape(128, 4, NCH, 128)).astype(np.float16)

    def vt5(x):
        # [mp, b, c, j]. even b: v in cols 0-63, ones col 64 (rv on PSUM
        # rows 0-63, den row 64). odd b: ones col 32 (32-aligned: walrus
        # rejects partition base 63), v in cols 64-127 (rv rows 64-127,
        # den row 32). Pair-packs a batch pair onto all 128 partitions.
        out = np.ones((128, NB, NCH, 65), np.float32)
        out[:, :, :, 0:64] = x.reshape(NB, 64, NCH, 128).transpose(3, 0, 2, 1)
        return out.astype(ml_dtypes.bfloat16)

    return {
        "q2": two(qh),
        "k2": two(kh),
        "qt": gramt(qh),
        "kt": gramt(kh),
        "vt": vt5(vh),
        "g_sim": np.asarray(gamma_sim[h], dtype=np.float32).reshape(1, 1),
        "gamma_val": np.asarray(
            gamma_val[h * C : (h + 1) * C], dtype=np.float32
        ).reshape(C, 1),
        "beta_val": np.asarray(
            beta_val[h * C : (h + 1) * C], dtype=np.float32
        ).reshape(C, 1),
    }


_CACHED_NC = None


def _setup_profiling():
    """Make run_bass_kernel_spmd(trace=True) work on images missing
    antenv.axon_hooks: inject the ctypes NTFF hook + keep artifacts local."""
    import contextlib
    import ctypes
    import types

    try:
        from antenv.axon_hooks import get_axon_ntff_profile_hook  # noqa: F401
    except ImportError:
        so_path = os.environ.get("AXON_PJRT_SO", "/opt/axon/libaxon_pjrt.so")
        lib = ctypes.CDLL(so_path)
        lib.axon_start_nrt_profile.argtypes = [
            ctypes.POINTER(ctypes.c_int64),
            ctypes.c_size_t,
        ]
        lib.axon_start_nrt_profile.restype = ctypes.c_int64
        lib.axon_stop_nrt_profile.argtypes = [ctypes.c_char_p]
        lib.axon_stop_nrt_profile.restype = ctypes.c_int64

        @contextlib.contextmanager
        def _hook(output_dir, device_ids):
            import jax

            jax.devices()
            if device_ids:
                ids = (ctypes.c_int64 * len(device_ids))(*device_ids)
                rc = lib.axon_start_nrt_profile(ids, len(device_ids))
            else:
                rc = lib.axon_start_nrt_profile(None, 0)
            if rc != 0:
                raise RuntimeError(f"axon_start_nrt_profile rc={rc}")
            try:
                yield
            finally:
                n = lib.axon_stop_nrt_profile(str(output_dir).encode())
                print(f"ntff profile: {n} file(s) -> {output_dir}", file=sys.stderr)

        mod = types.ModuleType("antenv.axon_hooks")
        mod.get_axon_ntff_profile_hook = lambda: _hook
        mod.set_axon_ntff_profile_hook = lambda h: None
        import antenv

        sys.modules["antenv.axon_hooks"] = mod
        antenv.axon_hooks = mod

    import concourse.bass_utils as bu

    bu.upload_artifacts = lambda tmpdir: f"local://{tmpdir}"


def kernel(query, key, value, gamma_sim, beta_sim, gamma_val, beta_val):
    global _CACHED_NC
    from concourse.bass_utils import run_bass_kernel_spmd

    query = np.asarray(query, dtype=np.float32)
    key = np.asarray(key, dtype=np.float32)
    value = np.asarray(value, dtype=np.float32)
    gamma_sim = np.asarray(gamma_sim, dtype=np.float32)
    gamma_val = np.asarray(gamma_val, dtype=np.float32)
    beta_val = np.asarray(beta_val, dtype=np.float32)

    if _CACHED_NC is None:
        _CACHED_NC = build_nc()
    nc = _CACHED_NC

    in_maps = [
        make_in_map(query, key, value, gamma_sim, None, gamma_val, beta_val, h)
        for h in range(8)
    ]
    trace = bool(int(os.environ.get("BASS_PROFILE", "0")))
    tmpdir = os.environ.get("BASS_PROFILE_DIR") or None
    if trace:
        try:
            _setup_profiling()
        except Exception as e:  # noqa: BLE001
            print(f"profiling setup failed ({e}); running untraced", file=sys.stderr)
            trace = False
    try:
        res = run_bass_kernel_spmd(
            nc, in_maps, list(range(8)), trace=trace, tmpdir=tmpdir
        )
    except Exception:
        if not trace:
            raise
        print("traced run failed; retrying untraced", file=sys.stderr)
        res = run_bass_kernel_spmd(nc, in_maps, list(range(8)), trace=False)
    if trace and res.exec_time_ns is not None:
        print(f"HW exec time: {res.exec_time_ns} ns")

    out = np.empty((NB, 8 * C, L), dtype=np.float32)
    for h in range(8):
        out[:, h * C : (h + 1) * C, :] = res.results[h]["out"]
    return out


# revision 27
# speedup vs baseline: 1.3769x; 1.0070x over previous
"""Trainium2 Bass kernel for nn_AttentionOperation_32521492365427.

kernel(**inputs) -> np.ndarray, full shapes:
  query/key/value: [8, 8, 64, 1024] f32; gamma_sim/beta_sim: [8];
  gamma_val/beta_val: [512]; output: [8, 512, 1024] f32.

Sharded by HEAD across the 8 NeuronCores (one head per core): both
BatchNorms then have core-local statistics, so there are no collectives.

Per-core math:
 - softmax is shift-invariant => the sim-BN reduces to one per-head scale
   s = gamma_sim / sqrt(var(logits) + EPS); beta/mean drop out (the
   mean^2 term of the variance is O(1e-8) of E[l^2] and is dropped).
 - sumsq(logits_b) = <Gq_b, Gk_b> over 64x64 Gram matrices; the grams of
   batch pairs (2p, 2p+1) are block-diagonal-packed into [128,128]
   matmuls (half the instructions of per-batch grams).
 - PV stationary is [128_m, 64 v | 1 ones]: PV row 64 is the softmax
   denominator; normalize = DVE copy+reciprocal of the den row, gpsimd
   partition-broadcast, DVE multiply straight out of PSUM.
 - val-BN affine + exact (erf) gelu fuse into one ACT pass per batch
   pair; s is broadcast to 128 partitions via a tiny ones-matmul.

Scheduling:
 - PE stream: warmup ramp -> pair-grams -> QK prefill -> bridge matmuls
   over the s-chain latency -> software-pipelined [QK(s+2), PV(s)]
   slots. The tensor engine clock (HAM) needs ~3us of continuous
   activity for 2.4 GHz; idle gaps halve it to 1.2 GHz.
 - QK row-group pairing: q/k are host-duplicated onto both partition
   halves, so the two 512-col j-halves of each QK chunk run in disjoint
   PE row groups (rows 0-63 / 64-127) and overlap in the array
   (measured start delta 3ns), halving QK wall time.
 - logits/pt live as [128,512] half-tiles (lg pool bufs=4 = 4 PSUM
   banks, pv double-buffered = 4 banks): the exp->QK write-after-read
   chain is half-granular so it never serializes the slot pipeline.
 - exp: ACT table exp for chunks {0,1,3,5,7}, DVE Schraudolph bf16-bits
   exp for {2,4,6} (logits reach +-43 sigma, so bf16 range is required).
 - per-batch drains (copy/recip/bcast/mul/bn_stats) pipeline inside the
   next batch's slots; odd batches drain to a staging tile that an idle
   DMA engine lifts to partitions 64-127, so gelu + output stores run
   128 partitions wide (full SDMA fanout) per batch pair. The last
   batch drains in quarter-granular pipelined steps.
 - gelu ACT table preloads via a dummy op gated on the BN aggregate;
   output DMAs alternate between the SP and ACT HWDGE queues.
"""

import os
import sys

sys.path.insert(0, "/opt/trn_rl_repo")

from contextlib import ExitStack

import numpy as np

import concourse.bacc as bacc
import concourse.bass as bass  # noqa: F401
import concourse.tile as tile
from concourse import mybir

F32 = mybir.dt.float32
F16 = mybir.dt.float16
BF16 = mybir.dt.bfloat16
I32 = mybir.dt.int32
I16 = mybir.dt.int16
AF = mybir.ActivationFunctionType
OP = mybir.AluOpType

EPS = 1e-3
NB = 8
D = 64
C = 64
L = 1024
M = 1024
NCH = M // 128
NS = NB * NCH
NLM = float(NB * L * M)
MAGIC = 0x5F3759DF
# DVE Schraudolph exp: bf16 bits = int16(x * 128/ln2 + 16250.49); ~3% max rel
# err, exact dynamic range to e^+-88. Offloaded chunks relieve the ACT engine.
SCHRAU_A = 184.6635
SCHRAU_B = 16250.49
DVE_EXP_CHUNKS = (2, 4, 6)


def _newton_rsqrt(nc, x, y, t, magic_i32, iters=1):
    """y = 1/sqrt(x) entirely on DVE (bit-trick seed + Newton iters)."""
    xi = x.bitcast(I32)
    yi = y.bitcast(I32)
    nc.vector.tensor_scalar(
        out=yi, in0=xi, scalar1=1, scalar2=None, op0=OP.arith_shift_right
    )
    nc.vector.tensor_tensor(out=yi, in0=magic_i32, in1=yi, op=OP.subtract)
    for _ in range(iters):
        nc.vector.tensor_mul(t, y, y)
        nc.vector.tensor_mul(t, t, x)
        nc.vector.tensor_scalar(
            out=t, in0=t, scalar1=-0.5, scalar2=1.5, op0=OP.mult, op1=OP.add
        )
        nc.vector.tensor_mul(y, y, t)


def _strip_of(inst):
    """(c0, c1) column-strip a PE weight-load/matmul touches, 32-rounded."""
    pos = getattr(inst, "tile_position", None) or (0, 0)
    size = getattr(inst, "tile_size", None)
    if size is None:
        cols = 128
    else:
        cols = size[1]
    cols = 32 * ((cols + 31) // 32)
    return (pos[1], min(128, pos[1] + cols))


def _merge_sync(dst, src_inst):
    si = src_inst.sync_info
    if si is None:
        return
    nsi = dst.sync_info
    if nsi is None:
        dst.sync_info = si
        return
    if si.on_wait:
        nsi.on_wait.extend(si.on_wait)
    if si.on_update:
        nsi.on_update.extend(si.on_update)
    dst.sync_info = nsi


def _optimize_ldweights(nc):
    """Per-col-strip ldweights dedupe + hoisting for col-tiled concurrency.

    1. Remove an InstLdweights whose (AP, perf/transpose/position) signature
       is already resident in every 32-col strip it covers (j-pairs and the
       odd-batch den/vt reloads). Its waits merge into the next matmul.
    2. Hoist an InstLdweights above an immediately-preceding InstMatmult
       whose col strip is disjoint (so back-to-back matmuls into disjoint
       col groups issue without an intervening weight load and overlap).
    """
    removed = hoisted = 0
    for bb in nc.m.functions[0].blocks:
        insts = list(bb.instructions)
        out = []
        resident = {}  # strip c0 -> (c1, sig)

        def clear_overlap(c0, c1):
            for k in list(resident):
                k1 = resident[k][0]
                if k < c1 and c0 < k1:
                    del resident[k]

        k = 0
        while k < len(insts):
            inst = insts[k]
            tname = type(inst).__name__
            if tname == "InstLdweights":
                ap = inst.ins[0]
                sig = (
                    str(ap),
                    getattr(inst, "perf_mode", None),
                    getattr(inst, "is_transpose", None),
                    getattr(inst, "tile_position", None),
                )
                c0, c1 = _strip_of(inst)
                ent = resident.get(c0)
                nxt = insts[k + 1] if k + 1 < len(insts) else None
                if (
                    ent is not None
                    and ent == (c1, sig)
                    and nxt is not None
                    and type(nxt).__name__ == "InstMatmult"
                ):
                    _merge_sync(nxt, inst)
                    removed += 1
                    k += 1
                    continue
                clear_overlap(c0, c1)
                resident[c0] = (c1, sig)
                # hoist above preceding disjoint-strip matmuls
                j = len(out)
                while j > 0 and type(out[j - 1]).__name__ == "InstMatmult":
                    mc0, mc1 = _strip_of(out[j - 1])
                    if mc0 < c1 and c0 < mc1:
                        break
                    j -= 1
                if j < len(out):
                    out.insert(j, inst)
                    hoisted += 1
                else:
                    out.append(inst)
                k += 1
                continue
            elif tname == "InstMatmult":
                pass  # matmuls do not disturb the loaded stationary
            elif getattr(inst, "engine", None) == mybir.EngineType.PE and not getattr(
                inst, "is_sequencer_only", False
            ):
                resident.clear()
            out.append(inst)
            k += 1
        bb.instructions = out
    return removed, hoisted


def build_nc(debug: bool = False):
    nc = bacc.Bacc("TRN2", target_bir_lowering=False, debug=debug)

    q2_d = nc.dram_tensor("q2", [128, NB, L], F16, kind="ExternalInput")
    k2_d = nc.dram_tensor("k2", [128, NB, L], F16, kind="ExternalInput")
    qt_d = nc.dram_tensor("qt", [128, NB // 2, NCH, 128], F16, kind="ExternalInput")
    kt_d = nc.dram_tensor("kt", [128, NB // 2, NCH, 128], F16, kind="ExternalInput")
    vt_d = nc.dram_tensor("vt", [128, NB, NCH, 65], BF16, kind="ExternalInput")
    gsim_d = nc.dram_tensor("g_sim", [1, 1], F32, kind="ExternalInput")
    gval_d = nc.dram_tensor("gamma_val", [C, 1], F32, kind="ExternalInput")
    bval_d = nc.dram_tensor("beta_val", [C, 1], F32, kind="ExternalInput")
    out_d = nc.dram_tensor("out", [NB, C, L], F32, kind="ExternalOutput")

    with tile.TileContext(nc) as tc, ExitStack() as ctx:
        const_p = ctx.enter_context(tc.tile_pool(name="const", bufs=1))
        pt_p = ctx.enter_context(tc.tile_pool(name="pt", bufs=6))
        rec_p = ctx.enter_context(tc.tile_pool(name="rec", bufs=2))
        pout_p = ctx.enter_context(tc.tile_pool(name="pout", bufs=4))
        small = ctx.enter_context(tc.tile_pool(name="small", bufs=1))
        lg_p = ctx.enter_context(tc.tile_pool(name="lg", bufs=4, space="PSUM"))

        # ---- constants ----
        warm_w = const_p.tile([128, 32], F16, tag="warmw")
        nc.vector.memset(warm_w[:], 1.0)
        warm_m = const_p.tile([128, 512], F16, tag="warmm")
        nc.vector.memset(warm_m[:], 1.0)
        ones_col = const_p.tile([128, 1], F32, tag="ones_col")
        nc.vector.memset(ones_col[:], 1.0)
        ones_row = const_p.tile([1, 128], F32, tag="ones_row")
        nc.vector.memset(ones_row[:], 1.0)
        magic1 = const_p.tile([1, 1], I32, tag="magic1")
        nc.vector.memset(magic1[:], MAGIC)
        magicC = const_p.tile([C, 1], I32, tag="magicC")
        nc.vector.memset(magicC[:], MAGIC)
        tiny = const_p.tile([1, 2], F32, tag="tiny")
        nc.vector.memset(tiny[:], 1.0)
        ones_bf = const_p.tile([128, 1], BF16, tag="ones_bf")
        nc.vector.memset(ones_bf[:], 1.0)

        # ACT exp table preload (first table load costs ~1.3us; hide at t=0)
        nc.scalar.activation(tiny[:, 1:2], tiny[:, 0:1], AF.Exp)

        gsim_sb = const_p.tile([1, 1], F32, tag="gsim")
        gval_sb = const_p.tile([C, 1], F32, tag="gval")
        bval_sb = const_p.tile([C, 1], F32, tag="bval")

        qt_sb = const_p.tile([128, NB // 2, NCH, 128], F16, tag="qt")
        kt_sb = const_p.tile([128, NB // 2, NCH, 128], F16, tag="kt")
        q2_sb = const_p.tile([128, NB, L], F16, tag="q2")
        k2_sb = const_p.tile([128, NB, L], F16, tag="k2")
        vt_sb = const_p.tile([128, NB, NCH, 65], BF16, tag="vt")

        # input DMAs, priority order. sync/scalar are HWDGE; vt on SWDGE.
        for p in range(4):
            nc.sync.dma_start(out=qt_sb[:, p], in_=qt_d[:, p])
            nc.scalar.dma_start(out=kt_sb[:, p], in_=kt_d[:, p])
        nc.sync.dma_start(out=q2_sb[:, 0:2], in_=q2_d[:, 0:2])
        nc.scalar.dma_start(out=k2_sb[:, 0:2], in_=k2_d[:, 0:2])
        nc.sync.dma_start(out=gsim_sb[:], in_=gsim_d[:])
        nc.sync.dma_start(out=gval_sb[:], in_=gval_d[:])
        nc.sync.dma_start(out=bval_sb[:], in_=bval_d[:])
        for g in range(1, 4):
            nc.sync.dma_start(out=q2_sb[:, 2 * g : 2 * g + 2], in_=q2_d[:, 2 * g : 2 * g + 2])
            nc.scalar.dma_start(out=k2_sb[:, 2 * g : 2 * g + 2], in_=k2_d[:, 2 * g : 2 * g + 2])
        for b in range(NB):
            nc.gpsimd.dma_start(out=vt_sb[:, b], in_=vt_d[:, b])

        s_bcast = small.tile([128, 1], F32, tag="sbc")
        sa_bcast = small.tile([128, 1], F32, tag="sabc")
        # pair-packed ue: even batch of pair g on partitions 0-63; odd
        # batches drain into a 64-partition staging tile that an (idle)
        # DMA engine lifts to partitions 64-127 for pair-wide gelu+store.
        ue_sb = const_p.tile([128, NB // 2, 2, 512], F32, tag="ue")
        statsA = small.tile([C, 2, NB // 2, 2 * 6], F32, tag="statsA")

        lg_tiles = {}
        pt_tiles = {}
        pv_tiles = {}
        rec_tiles = {}

        def emit_qk(s):
            # q/k are host-duplicated on both partition halves: the j0 half
            # contracts array rows 0-63 and j1 rows 64-127 (disjoint row
            # groups), so the two 512-col matmuls overlap in the PE array.
            b, c = divmod(s, NCH)
            for j in range(2):
                rs = slice(64 * j, 64 * j + 64)
                lg = lg_p.tile([128, 512], F32, tag="lg", name="lg")
                nc.tensor.matmul(
                    lg[:],
                    k2_sb[rs, b, c * 128 : (c + 1) * 128],
                    q2_sb[rs, b, j * 512 : (j + 1) * 512],
                    start=True,
                    stop=True,
                )
                lg_tiles[(s, j)] = lg

        def emit_exp(s):
            c = s % NCH
            for j in range(2):
                lg = lg_tiles.pop((s, j))
                pt = pt_p.tile([128, 512], BF16, tag="pt", name="pt")
                if c in DVE_EXP_CHUNKS:
                    nc.vector.tensor_scalar(
                        out=pt.bitcast(I16)[:],
                        in0=lg[:],
                        scalar1=sa_bcast[:, 0:1],
                        scalar2=SCHRAU_B,
                        op0=OP.mult,
                        op1=OP.add,
                    )
                else:
                    nc.scalar.activation(pt[:], lg[:], AF.Exp, scale=s_bcast[:, 0:1])
                pt_tiles[(s, j)] = pt

        # ---- phase 0: warmup ramp + pair-gram variance -> s ----
        with tc.tile_pool(name="warm", bufs=1, space="PSUM") as warm_p, tc.tile_pool(
            name="gram", bufs=1, space="PSUM"
        ) as gram_p, tc.tile_pool(name="vs", bufs=1, space="PSUM") as vs_p:
            wps = warm_p.tile([32, 512], F32, tag="warm")
            for _ in range(6):
                nc.tensor.matmul(wps[:], warm_w[:], warm_m[:], start=True, stop=True)

            acc = small.tile([128, 4], F32, tag="acc")
            for p in range(4):
                gq_ps = gram_p.tile([128, 128], F32, tag="gq")
                gk_ps = gram_p.tile([128, 128], F32, tag="gk")
                for c in range(NCH):
                    nc.tensor.matmul(
                        gq_ps[:],
                        qt_sb[:, p, c],
                        qt_sb[:, p, c],
                        start=(c == 0),
                        stop=(c == NCH - 1),
                    )
                for c in range(NCH):
                    nc.tensor.matmul(
                        gk_ps[:],
                        kt_sb[:, p, c],
                        kt_sb[:, p, c],
                        start=(c == 0),
                        stop=(c == NCH - 1),
                    )
                gq_sb = small.tile([128, 128], F32, tag="gq_sb")
                nc.vector.tensor_copy(gq_sb[:], gq_ps[:])
                prod = small.tile([128, 128], F32, tag="prod")
                nc.vector.tensor_mul(prod[:], gq_sb[:], gk_ps[:])
                # only the block-diagonal quadrants are per-batch grams
                nc.vector.reduce_sum(
                    acc[0:64, p : p + 1], prod[0:64, 0:64], axis=mybir.AxisListType.X
                )
                nc.vector.reduce_sum(
                    acc[64:128, p : p + 1],
                    prod[64:128, 64:128],
                    axis=mybir.AxisListType.X,
                )

            # QK prefill while the s-chain resolves
            emit_qk(0)
            emit_qk(1)

            red = small.tile([128, 1], F32, tag="red")
            nc.vector.reduce_sum(red[:], acc[:], axis=mybir.AxisListType.X)
            vs_ps = vs_p.tile([128, 2], F32, tag="vs")
            var_ps = vs_ps[0:1, 0:1]
            sb_ps = vs_ps[:, 1:2]
            nc.tensor.matmul(var_ps, ones_col[:], red[:], start=True, stop=True)

            sv = small.tile([1, 6], F32, tag="sv")
            nc.vector.tensor_scalar(
                out=sv[:, 0:1],
                in0=var_ps,
                scalar1=1.0 / NLM,
                scalar2=EPS,
                op0=OP.mult,
                op1=OP.add,
            )
            _newton_rsqrt(nc, sv[:, 0:1], sv[:, 1:2], sv[:, 2:3], magic1[:])
            nc.vector.tensor_mul(sv[:, 3:4], sv[:, 1:2], gsim_sb[:])
            nc.tensor.matmul(sb_ps, ones_row[:], sv[:, 3:4], start=True, stop=True)
            nc.vector.tensor_copy(s_bcast[:], sb_ps)
            nc.vector.tensor_scalar_mul(sa_bcast[:], s_bcast[:], SCHRAU_A)
            # bridge chain stage 2: gated on s_bcast, covers exp(0) latency
            nc.tensor.matmul(
                wps[0:1, 0:1], ones_col[:], s_bcast[:], start=True, stop=True
            )
            for _ in range(3):
                nc.tensor.matmul(
                    wps[:], warm_w[:], warm_m[:], start=True, stop=True
                )

            # free-floating bridge matmuls (scheduler places them)
            for _ in range(6):
                nc.tensor.matmul(
                    wps[:], warm_w[:], warm_m[:], start=True, stop=True
                )

        emit_exp(0)

        # ---- phase A: software-pipelined [QK(s+2), PV(s)] slots ----
        den_tiles = {}
        stage_tiles = {}

        def drain_copy(b, lo, hi):
            if b not in den_tiles:
                den_tiles[b] = rec_p.tile([1, L], F32, tag="den0", name="den0")
            nc.vector.tensor_copy(den_tiles[b][:, lo:hi], pv_tiles[b][64:65, lo:hi])

        def drain_recip(b, lo, hi):
            nc.vector.reciprocal_approx_fast(
                out=den_tiles[b][:, lo:hi], in_=den_tiles[b][:, lo:hi]
            )

        def drain_bcast(b, lo, hi):
            if b not in rec_tiles:
                rec_tiles[b] = rec_p.tile([C, L], F32, tag="rec", name="rec")
            nc.gpsimd.partition_broadcast(
                rec_tiles[b][:, lo:hi], den_tiles[b][:, lo:hi]
            )

        def drain_mul(b, lo, hi):
            g = b // 2
            hs = slice(lo // 512, (hi + 511) // 512)
            if b % 2 == 0:
                out = ue_sb[0:64, g, hs, :]
            else:
                if b not in stage_tiles:
                    stage_tiles[b] = rec_p.tile(
                        [C, 2, 512], F32, tag="stage", name="stage"
                    )
                out = stage_tiles[b][:, hs, :]
            nc.vector.tensor_mul(
                out, pv_tiles[b][0:C, lo:hi], rec_tiles[b][:, lo:hi]
            )

        def drain_lift(b):
            # idle DMA engines lift the odd batch to partitions 64-127
            g = b // 2
            nc.sync.dma_start(out=ue_sb[64:128, g], in_=stage_tiles[b][:])

        def drain_stats(b, h):
            g, par = divmod(b, 2)
            srcap = (
                ue_sb[0:64, g, h, :] if par == 0 else stage_tiles[b][:, h, :]
            )
            nc.vector.bn_stats(statsA[:, par, g, h * 6 : (h + 1) * 6], srcap)

        with tc.tile_pool(name="pv", bufs=2, space="PSUM") as pv_p:
            for s in range(NS):
                b, c = divmod(s, NCH)
                if s + 1 < NS:
                    emit_exp(s + 1)
                if c == 0:
                    pv_tiles[b] = pv_p.tile([128, L], F32, tag="pv", name="pv")
                if s + 2 < NS:
                    emit_qk(s + 2)
                pv = pv_tiles[b]
                for j in range(2):
                    pt = pt_tiles.pop((s, j))
                    nc.tensor.matmul(
                        pv[0:65, j * 512 : (j + 1) * 512],
                        vt_sb[:, b, c, :],
                        pt[:],
                        start=(c == 0),
                        stop=(c == NCH - 1),
                        skip_group_check=True,
                    )
                if b >= 1:
                    bb = b - 1
                    if c == 1:
                        drain_copy(bb, 0, L)
                        drain_recip(bb, 0, L)
                    elif c == 2:
                        drain_bcast(bb, 0, L)
                    elif c == 3:
                        drain_mul(bb, 0, L)
                    elif c == 4:
                        drain_stats(bb, 0)
                        if bb % 2 == 1:
                            drain_lift(bb)
                    elif c == 5:
                        drain_stats(bb, 1)
                        pv_tiles.pop(bb)
                        rec_tiles.pop(bb)
                        den_tiles.pop(bb)

            # last batch drains in pipelined quarters (DVE chain) with
            # the gpsimd broadcasts and stats trailing per half
            for q in range(4):
                drain_copy(NB - 1, q * 256, (q + 1) * 256)
                drain_recip(NB - 1, q * 256, (q + 1) * 256)
                if q % 2 == 1:
                    drain_bcast(NB - 1, (q - 1) * 256, (q + 1) * 256)
                    drain_mul(NB - 1, (q - 1) * 256, (q + 1) * 256)
                    drain_stats(NB - 1, q // 2)
            drain_lift(NB - 1)
            pv_tiles.pop(NB - 1)
            rec_tiles.pop(NB - 1)
            den_tiles.pop(NB - 1)

        # ---- phase B: val-BN affine (64-wide), ab lifted to 128, pair-wide
        # gelu + full-fanout pair stores ----
        chan = small.tile([C, 2], F32, tag="chan")
        nc.vector.bn_aggr(chan[:], statsA[:])
        # gelu ACT table preload: dep on chan keeps it post-drains; the
        # 1.5us table load overlaps the affine chain below
        nc.scalar.activation(tiny[:, 1:2], chan[0:1, 0:1], AF.Gelu)
        vb = small.tile([C, 6], F32, tag="vb")
        nc.vector.tensor_scalar_add(vb[:, 0:1], chan[:, 1:2], EPS)
        _newton_rsqrt(nc, vb[:, 0:1], vb[:, 1:2], vb[:, 2:3], magicC[:])
        ab = small.tile([128, 2], F32, tag="ab")
        nc.vector.tensor_mul(ab[0:64, 0:1], gval_sb[:], vb[:, 1:2])
        nc.vector.tensor_mul(vb[:, 3:4], chan[:, 0:1], ab[0:64, 0:1])
        nc.vector.tensor_sub(ab[0:64, 1:2], bval_sb[:], vb[:, 3:4])
        nc.vector.stream_shuffle(ab[64:128, :], ab[0:64, :], mask=list(range(32)))

        for g in range(4):
            pout = pout_p.tile([128, 2, 512], F32, tag="pout")
            nc.scalar.activation(
                pout[:],
                ue_sb[:, g],
                AF.Gelu,
                scale=ab[:, 0:1],
                bias=ab[:, 1:2],
            )
            eng = nc.sync if g % 2 == 0 else nc.scalar
            eng.dma_start(out=out_d[2 * g : 2 * g + 2], in_=pout[:])

    n, h = _optimize_ldweights(nc)
    print(f"deduped {n} ldweights, hoisted {h}", file=sys.stderr)
    nc.compile()
    return nc


def make_in_map(q, k, v, gamma_sim, beta_sim, gamma_val, beta_val, h):
    """Build the per-core (per-head) input map. Layout-only host prep."""
    import ml_dtypes

    qh = q[:, h]
    kh = k[:, h]
    vh = v[:, h]

    def two(x):
        # [p, b, l] <- x[b, p % 64, l]: each batch duplicated on both
        # partition halves so QK j-halves can use disjoint PE row groups
        t = x.transpose(1, 0, 2)
        return np.ascontiguousarray(
            np.concatenate([t, t], axis=0)
        ).astype(np.float16)

    def gramt(x):
        # [lp, pair, c, j=r*64+d] <- x[2*pair+r, d, c*128+lp]
        t = x.reshape(4, 2, 64, NCH, 128).transpose(4, 0, 3, 1, 2)
        return np.ascontiguousarray(t.reshape(128, 4, NCH, 128)).astype(np.float16)

    def vt5(x):
        # [mp, b, c, j]. even b: v in cols 0-63, ones col 64 (rv on PSUM
        # rows 0-63, den row 64). odd b: ones col 32 (32-aligned: walrus
        # rejects partition base 63), v in cols 64-127 (rv rows 64-127,
        # den row 32). Pair-packs a batch pair onto all 128 partitions.
        out = np.ones((128, NB, NCH, 65), np.float32)
        out[:, :, :, 0:64] = x.reshape(NB, 64, NCH, 128).transpose(3, 0, 2, 1)
        return out.astype(ml_dtypes.bfloat16)

    return {
        "q2": two(qh),
        "k2": two(kh),
        "qt": gramt(qh),
        "kt": gramt(kh),
        "vt": vt5(vh),
        "g_sim": np.asarray(gamma_sim[h], dtype=np.float32).reshape(1, 1),
        "gamma_val": np.asarray(
            gamma_val[h * C : (h + 1) * C], dtype=np.float32
        ).reshape(C, 1),
        "beta_val": np.asarray(
            beta_val[h * C : (h + 1) * C], dtype=np.float32
        ).reshape(C, 1),
    }


_CACHED_NC = None


def _setup_profiling():
    """Make run_bass_kernel_spmd(trace=True) work on images missing
    antenv.axon_hooks: inject the ctypes NTFF hook + keep artifacts local."""
    import contextlib
    import ctypes
    import types

    try:
        from antenv.axon_hooks import get_axon_ntff_profile_hook  # noqa: F401
    except ImportError:
        so_path = os.environ.get("AXON_PJRT_SO", "/opt/axon/libaxon_pjrt.so")
        lib = ctypes.CDLL(so_path)
        lib.axon_start_nrt_profile.argtypes = [
            ctypes.POINTER(ctypes.c_int64),
            ctypes.c_size_t,
        ]
        lib.axon_start_nrt_profile.restype = ctypes.c_int64
        lib.axon_stop_nrt_profile.argtypes = [ctypes.c_char_p]
        lib.axon_stop_nrt_profile.restype = ctypes.c_int64

        @contextlib.contextmanager
        def _hook(output_dir, device_ids):
            import jax

            jax.devices()
            if device_ids:
                ids = (ctypes.c_int64 * len(device_ids))(*device_ids)
                rc = lib.axon_start_nrt_profile(ids, len(device_ids))
            else:
                rc = lib.axon_start_nrt_profile(None, 0)
            if rc != 0:
                raise RuntimeError(f"axon_start_nrt_profile rc={rc}")
            try:
                yield
            finally:
                n = lib.axon_stop_nrt_profile(str(output_dir).encode())
                print(f"ntff profile: {n} file(s) -> {output_dir}", file=sys.stderr)

        mod = types.ModuleType("antenv.axon_hooks")
        mod.get_axon_ntff_profile_hook = lambda: _hook
        mod.set_axon_ntff_profile_hook = lambda h: None
        import antenv

        sys.modules["antenv.axon_hooks"] = mod
        antenv.axon_hooks = mod

    import concourse.bass_utils as bu

    bu.upload_artifacts = lambda tmpdir: f"local://{tmpdir}"


def kernel(query, key, value, gamma_sim, beta_sim, gamma_val, beta_val):
    global _CACHED_NC
    from concourse.bass_utils import run_bass_kernel_spmd

    query = np.asarray(query, dtype=np.float32)
    key = np.asarray(key, dtype=np.float32)
    value = np.asarray(value, dtype=np.float32)
    gamma_sim = np.asarray(gamma_sim, dtype=np.float32)
    gamma_val = np.asarray(gamma_val, dtype=np.float32)
    beta_val = np.asarray(beta_val, dtype=np.float32)

    if _CACHED_NC is None:
        _CACHED_NC = build_nc()
    nc = _CACHED_NC

    in_maps = [
        make_in_map(query, key, value, gamma_sim, None, gamma_val, beta_val, h)
        for h in range(8)
    ]
    trace = bool(int(os.environ.get("BASS_PROFILE", "0")))
    tmpdir = os.environ.get("BASS_PROFILE_DIR") or None
    if trace:
        try:
            _setup_profiling()
        except Exception as e:  # noqa: BLE001
            print(f"profiling setup failed ({e}); running untraced", file=sys.stderr)
            trace = False
    try:
        res = run_bass_kernel_spmd(
            nc, in_maps, list(range(8)), trace=trace, tmpdir=tmpdir
        )
    except Exception:
        if not trace:
            raise
        print("traced run failed; retrying untraced", file=sys.stderr)
        res = run_bass_kernel_spmd(nc, in_maps, list(range(8)), trace=False)
    if trace and res.exec_time_ns is not None:
        print(f"HW exec time: {res.exec_time_ns} ns")

    out = np.empty((NB, 8 * C, L), dtype=np.float32)
    for h in range(8):
        out[:, h * C : (h + 1) * C, :] = res.results[h]["out"]
    return out
